# revision 11
# baseline (speedup 1.0000x reference)
"""Multi-head attention (B=2, S=2048, D=1024, H=16 heads, causal) on 8 TRN2
NeuronCores, head-parallel: each core computes 2 heads' Q/K/V projections,
attention, and a partial output projection (its 128-row slice of Wo); the
host sums the 8 partial outputs.

Per-core layout (matmul operands bf16, fp32 PSUM accumulation):
  - xT [1024, 4096]      x transposed, shared by all cores
  - wq/wk/wv [1024,128]  per-core column slice of Wq/Wk/Wv
  - wo [128, 1024]       per-core row slice of Wo
  - msk [128, 896]       sliding causal mask: msk[i, c] = 1 if c-384 >= i
  QT/KT are produced transposed [128 = 2 heads x 64 head dims, 4096 tokens];
  V is produced natural per (b, h, kv-tile) as [128 kv, 64] with an appended
  ones column so the attention matmul also accumulates softmax denominators
  (row 64 of the [65, 512] PSUM output).

Emission is interleaved at "unit" granularity: while strip g's attention
(ACT-heavy) is emitted, strip g+1's projection matmuls and strip g-1's
output-projection matmuls are interspersed so the PE never idles long
enough for the HAM clock gate to re-throttle.
"""

import numpy as np
import ml_dtypes
from contextlib import ExitStack

import concourse.bass as bass
import concourse.bacc as bacc
import concourse.tile as tile
import concourse.mybir as mybir
from concourse.bass_utils import run_bass_kernel_spmd

BF16 = mybir.dt.bfloat16
F32 = mybir.dt.float32
NPBF16 = ml_dtypes.bfloat16

D = 1024          # model dim
B = 2
S = 2048
NT = B * S        # 4096 flattened tokens
HD = 64           # head dim
H = 16            # total heads
NCORES = 8
HLOC = H // NCORES  # 2 heads per core
CW = HLOC * HD      # 128 local columns
QSTRIP = 512
NSTRIP = NT // QSTRIP  # 8 strips
KT_TILES = S // 128    # 16 kv tiles per batch


def _interleave(main, fill):
    """Emit main units with fill units spread proportionally between them."""
    n, m = len(main), len(fill)
    if n == 0:
        for u in fill:
            u()
        return
    fi = 0
    for i, u in enumerate(main):
        u()
        tgt = ((i + 1) * m) // n
        while fi < tgt:
            fill[fi]()
            fi += 1
    while fi < m:
        fill[fi]()
        fi += 1


def _build_kernel(ctx: ExitStack, tc: tile.TileContext):
    nc = tc.nc
    xt = nc.dram_tensor("xt", [D, NT], BF16, kind="ExternalInput").ap()
    wq = nc.dram_tensor("wq", [D, CW], BF16, kind="ExternalInput").ap()
    wk = nc.dram_tensor("wk", [D, CW], BF16, kind="ExternalInput").ap()
    wv = nc.dram_tensor("wv", [D, CW], BF16, kind="ExternalInput").ap()
    wo = nc.dram_tensor("wo", [CW, D], BF16, kind="ExternalInput").ap()
    msk = nc.dram_tensor("msk", [128, 896], BF16, kind="ExternalInput").ap()
    out = nc.dram_tensor("out", [NT, D], F32, kind="ExternalOutput").ap()

    singles = ctx.enter_context(tc.tile_pool(name="singles", bufs=1))
    sbp = ctx.enter_context(tc.tile_pool(name="sbp", bufs=2))
    expp = ctx.enter_context(tc.tile_pool(name="expp", bufs=6))
    outp = ctx.enter_context(tc.tile_pool(name="outp", bufs=3))
    psA = ctx.enter_context(tc.tile_pool(name="psA", bufs=2, space="PSUM"))
    psS = ctx.enter_context(tc.tile_pool(name="psS", bufs=2, space="PSUM"))
    psV = ctx.enter_context(tc.tile_pool(name="psV", bufs=2, space="PSUM"))
    psO = ctx.enter_context(tc.tile_pool(name="psO", bufs=2, space="PSUM"))
    drp = ctx.enter_context(tc.tile_pool(name="drp", bufs=2, space="DRAM"))

    # --- staging: small weights first, then xT strip-major so strip 0's
    # projection can start ~2us in while later strips stream in behind it.
    w_sb = {}
    xt_sb = singles.tile([128, 8, NT], BF16)

    def load_w(name, w):
        t = singles.tile([128, 8, CW], BF16, tag=f"w{name}", name=f"w_{name}")
        nc.sync.dma_start(out=t, in_=w.rearrange("(k p) c -> p k c", p=128))
        w_sb[name] = t

    def load_xt(g):
        gs = g * QSTRIP
        nc.sync.dma_start(
            out=xt_sb[:, :, gs:gs + QSTRIP],
            in_=xt[:, gs:gs + QSTRIP].rearrange("(k p) c -> p k c", p=128))

    load_w("q", wq)
    load_xt(0)
    load_xt(1)
    load_w("k", wk)
    load_w("v", wv)
    msk_sb = singles.tile([128, 896], BF16)
    nc.sync.dma_start(out=msk_sb, in_=msk)
    wo_sb = singles.tile([128, D], BF16)
    nc.sync.dma_start(out=wo_sb, in_=wo)
    for g in range(2, NSTRIP):
        load_xt(g)

    qt_sb = singles.tile([128, NT], BF16)
    kt_sb = singles.tile([128, NT], BF16)
    v_sb = singles.tile([128, B * HLOC * KT_TILES, HD + 1], BF16)
    nc.vector.memset(v_sb[:, :, HD:HD + 1], 1.0)

    avf = {}  # strip -> assembled [128, 512] bf16 avT tile (both heads)

    def proj_units(g):
        gs = g * QSTRIP
        st = {}
        units = []

        def qk_mm(name, lo, hi, first, last, dst):
            def u():
                if first:
                    st[name] = psA.tile([128, QSTRIP], F32, tag="proj", name=f"ps_{name}")
                ps = st[name]
                for k in range(lo, hi):
                    nc.tensor.matmul(
                        ps, lhsT=w_sb[name][:, k, :],
                        rhs=xt_sb[:, k, gs:gs + QSTRIP],
                        start=(k == 0), stop=(k == 7))
                if last:
                    nc.vector.tensor_copy(dst[:, gs:gs + QSTRIP], ps)
            return u

        units.append(qk_mm("q", 0, 4, True, False, qt_sb))
        units.append(qk_mm("q", 4, 8, False, True, qt_sb))
        units.append(qk_mm("k", 0, 4, True, False, kt_sb))
        units.append(qk_mm("k", 4, 8, False, True, kt_sb))

        def v_mm(tt):
            b, j = divmod(g, 4)

            def u():
                if tt == 0:
                    st["v"] = psA.tile([128, QSTRIP], F32, tag="proj", name="ps_v")
                ps = st["v"]
                for k in range(8):
                    nc.tensor.matmul(
                        ps[:, tt * 128:(tt + 1) * 128],
                        lhsT=xt_sb[:, k, gs + tt * 128:gs + (tt + 1) * 128],
                        rhs=w_sb["v"][:, k, :],
                        start=(k == 0), stop=(k == 7))
                # both heads' v slices in one strided copy; idx h-stride = 16
                idx = b * HLOC * KT_TILES + 4 * j + tt
                nc.vector.tensor_copy(
                    v_sb[:, idx:idx + KT_TILES + 1:KT_TILES, 0:HD],
                    v_ps_view(ps, tt))
            return u

        def v_ps_view(ps, tt):
            return ps[:, tt * 128:(tt + 1) * 128].rearrange(
                "p (h d) -> p h d", h=2)

        for tt in range(4):
            units.append(v_mm(tt))
        return units

    def attn_units(g):
        b, j = divmod(g, 4)
        units = []
        st = {}

        def mk_tile(h, t):
            def u():
                if t == 0:
                    if h == 0:
                        avf[g] = sbp.tile([128, QSTRIP], BF16, tag="avf", name="avf")
                    st["av"] = psV.tile([HD + 1, QSTRIP], F32, tag="av", name="av_ps")
                av_ps = st["av"]
                hp = h * HD
                ntl = 4 * (j + 1)
                r = t - 4 * j
                q0 = 128 * r if r > 0 else 0  # valid q range start (causal)
                sc_ps = psS.tile([128, QSTRIP], F32, tag="sc")
                nc.tensor.matmul(
                    sc_ps[:, q0:],
                    lhsT=kt_sb[hp:hp + HD, b * S + t * 128: b * S + (t + 1) * 128],
                    rhs=qt_sb[hp:hp + HD,
                              b * S + j * QSTRIP + q0: b * S + (j + 1) * QSTRIP],
                    start=True, stop=True)
                pexp = expp.tile([128, QSTRIP], BF16, tag="pexp")
                nc.scalar.activation(
                    pexp[:, q0:], sc_ps[:, q0:],
                    mybir.ActivationFunctionType.Exp, scale=0.125)
                if r >= 0:  # triangular mask on the 128-wide diagonal block
                    nc.vector.tensor_mul(
                        pexp[:, q0:q0 + 128], pexp[:, q0:q0 + 128],
                        msk_sb[:, 384:512])
                idx = (b * HLOC + h) * KT_TILES + t
                nc.tensor.matmul(
                    av_ps[:, q0:], lhsT=v_sb[:, idx, :], rhs=pexp[:, q0:],
                    start=(t == 0), stop=(t == ntl - 1))
            return u

        def mk_norm(h):
            def u():
                av_ps = st["av"]
                s_sb = sbp.tile([HD + 1, QSTRIP], F32, tag="s", name="s_sb")
                nc.vector.tensor_copy(s_sb[0:HD, :], av_ps[0:HD, :])
                nc.vector.tensor_copy(s_sb[HD:HD + 1, :], av_ps[HD:HD + 1, :])
                s_dr = drp.tile([1, QSTRIP], F32, tag="sdr")
                nc.sync.dma_start(out=s_dr, in_=s_sb[HD:HD + 1, :])
                rb = sbp.tile([HD, QSTRIP], F32, tag="rb")
                nc.sync.dma_start(
                    out=rb, in_=s_dr[0, :].partition_broadcast(HD))
                nc.vector.reciprocal_approx_fast(rb, rb)
                avh = sbp.tile([HD, QSTRIP], BF16, tag="avh")
                nc.vector.tensor_mul(avh, s_sb[0:HD, :], rb)
                nc.sync.dma_start(out=avf[g][h * HD:(h + 1) * HD, :], in_=avh)
            return u

        for h in range(HLOC):
            for t in range(4 * (j + 1)):
                units.append(mk_tile(h, t))
            units.append(mk_norm(h))
        return units

    def out_units(g):
        gs = g * QSTRIP
        units = []

        def mk(tt):
            def u():
                ob = outp.tile([128, D], F32, tag="ob")
                for n in range(2):
                    op_ps = psO.tile([128, 512], F32, tag="op")
                    nc.tensor.matmul(
                        op_ps, lhsT=avf[g][:, tt * 128:(tt + 1) * 128],
                        rhs=wo_sb[:, n * 512:(n + 1) * 512],
                        start=True, stop=True)
                    nc.vector.tensor_copy(ob[:, n * 512:(n + 1) * 512], op_ps)
                nc.sync.dma_start(
                    out=out[gs + tt * 128: gs + (tt + 1) * 128, :], in_=ob)
            return u
        for tt in range(4):
            units.append(mk(tt))
        return units

    order = [0, 1, 2, 3, 7, 6, 5, 4]
    proj_fill = {0: [1], 1: [2], 2: [3, 4], 3: [5, 6, 7],
                 7: [], 6: [], 5: [], 4: []}
    for u in proj_units(0):
        u()
    prev = None
    for g in order:
        fill = []
        for pg in proj_fill[g]:
            fill += proj_units(pg)
        if prev is not None:
            fill += out_units(prev)
        _interleave(attn_units(g), fill)
        prev = g
    for u in out_units(order[-1]):
        u()


_CACHED_NC = None


def build_module():
    global _CACHED_NC
    if _CACHED_NC is None:
        nc = bacc.Bacc("TRN2", debug=False)
        with tile.TileContext(nc) as tc:
            with ExitStack() as ctx:
                _build_kernel(ctx, tc)
        nc.compile()
        _CACHED_NC = nc
    return _CACHED_NC


def make_in_maps(x, Wq, Wk, Wv, Wo):
    x = np.asarray(x, np.float32)
    xT = np.ascontiguousarray(x.reshape(NT, D).T).astype(NPBF16)
    # sliding causal mask: keep (c - 384) >= i
    i = np.arange(128)[:, None]
    c = np.arange(896)[None, :]
    msk = ((c - 384) >= i).astype(NPBF16)
    in_maps = []
    for core in range(NCORES):
        cs = slice(core * CW, (core + 1) * CW)
        in_maps.append({
            "xt": xT,
            "wq": np.asarray(Wq, np.float32)[:, cs].astype(NPBF16),
            "wk": np.asarray(Wk, np.float32)[:, cs].astype(NPBF16),
            "wv": np.asarray(Wv, np.float32)[:, cs].astype(NPBF16),
            "wo": np.ascontiguousarray(np.asarray(Wo, np.float32)[cs, :]).astype(NPBF16),
            "msk": msk,
        })
    return in_maps


def kernel(x, Wq, bq, Wk, bk, Wv, bv, Wo, bo):
    for b_ in (bq, bk, bv, bo):
        assert np.count_nonzero(np.asarray(b_)) == 0, "nonzero biases unsupported"
    nc = build_module()
    in_maps = make_in_maps(x, Wq, Wk, Wv, Wo)
    res = run_bass_kernel_spmd(nc, in_maps, core_ids=list(range(NCORES)))
    partials = [res.results[c]["out"] for c in range(NCORES)]
    total = np.sum(np.stack(partials, 0), axis=0, dtype=np.float32)
    return total.reshape(B, S, D)


# revision 13
# speedup vs baseline: 1.1657x; 1.1657x over previous
"""Multi-head attention (B=2, S=2048, D=1024, H=16 heads, causal) on 8 TRN2
NeuronCores, head-parallel: each core computes 2 heads' Q/K/V projections,
attention, and a partial output projection (its 128-row slice of Wo); the
host sums the 8 partial outputs.

Per-core layout (matmul operands bf16, fp32 PSUM accumulation):
  - xT [1024, 4096]      x transposed, shared by all cores
  - wq/wk/wv [1024,128]  per-core column slice of Wq/Wk/Wv
  - wo [128, 1024]       per-core row slice of Wo
  - msk [128, 896]       sliding causal mask: msk[i, c] = 1 if c-384 >= i
  QT/KT are produced transposed [128 = 2 heads x 64 head dims, 4096 tokens];
  V is produced natural per (b, h, kv-tile) as [128 kv, 64] with an appended
  ones column so the attention matmul also accumulates softmax denominators
  (row 64 of the [65, 512] PSUM output).

Emission is interleaved at "unit" granularity: while strip g's attention
(ACT-heavy) is emitted, strip g+1's projection matmuls and strip g-1's
output-projection matmuls are interspersed so the PE never idles long
enough for the HAM clock gate to re-throttle.
"""

import numpy as np
import ml_dtypes
from contextlib import ExitStack

import concourse.bass as bass
import concourse.bacc as bacc
import concourse.tile as tile
import concourse.mybir as mybir
from concourse.bass_utils import run_bass_kernel_spmd

BF16 = mybir.dt.bfloat16
F32 = mybir.dt.float32
NPBF16 = ml_dtypes.bfloat16

D = 1024          # model dim
B = 2
S = 2048
NT = B * S        # 4096 flattened tokens
HD = 64           # head dim
H = 16            # total heads
NCORES = 8
HLOC = H // NCORES  # 2 heads per core
CW = HLOC * HD      # 128 local columns
QSTRIP = 512
NSTRIP = NT // QSTRIP  # 8 strips
KT_TILES = S // 128    # 16 kv tiles per batch


def _interleave(main, fill):
    """Emit main units with fill units spread proportionally between them."""
    n, m = len(main), len(fill)
    if n == 0:
        for u in fill:
            u()
        return
    fi = 0
    for i, u in enumerate(main):
        u()
        tgt = ((i + 1) * m) // n
        while fi < tgt:
            fill[fi]()
            fi += 1
    while fi < m:
        fill[fi]()
        fi += 1


def _build_kernel(ctx: ExitStack, tc: tile.TileContext):
    nc = tc.nc
    xt = nc.dram_tensor("xt", [D, NT], BF16, kind="ExternalInput").ap()
    wq = nc.dram_tensor("wq", [D, CW], BF16, kind="ExternalInput").ap()
    wk = nc.dram_tensor("wk", [D, CW], BF16, kind="ExternalInput").ap()
    wv = nc.dram_tensor("wv", [D, CW], BF16, kind="ExternalInput").ap()
    wo = nc.dram_tensor("wo", [CW, D], BF16, kind="ExternalInput").ap()
    msk = nc.dram_tensor("msk", [128, 896], BF16, kind="ExternalInput").ap()
    out = nc.dram_tensor("out", [NT, D], F32, kind="ExternalOutput").ap()

    singles = ctx.enter_context(tc.tile_pool(name="singles", bufs=1))
    sbp = ctx.enter_context(tc.tile_pool(name="sbp", bufs=2))
    expp = ctx.enter_context(tc.tile_pool(name="expp", bufs=6))
    outp = ctx.enter_context(tc.tile_pool(name="outp", bufs=3))
    psM = ctx.enter_context(tc.tile_pool(name="psM", bufs=2, space="PSUM"))
    psS = ctx.enter_context(tc.tile_pool(name="psS", bufs=2, space="PSUM"))
    psV = ctx.enter_context(tc.tile_pool(name="psV", bufs=2, space="PSUM"))
    drp = ctx.enter_context(tc.tile_pool(name="drp", bufs=2, space="DRAM"))

    # --- staging: small weights first, then xT strip-major so strip 0's
    # projection can start ~2us in while later strips stream in behind it.
    w_sb = {}
    xt_sb = singles.tile([128, 8, NT], BF16)

    def load_w(name, w):
        t = singles.tile([128, 8, CW], BF16, tag=f"w{name}", name=f"w_{name}")
        nc.sync.dma_start(out=t, in_=w.rearrange("(k p) c -> p k c", p=128))
        w_sb[name] = t

    def load_xt(g):
        gs = g * QSTRIP
        nc.sync.dma_start(
            out=xt_sb[:, :, gs:gs + QSTRIP],
            in_=xt[:, gs:gs + QSTRIP].rearrange("(k p) c -> p k c", p=128))

    load_w("q", wq)
    load_xt(0)
    load_xt(1)
    load_w("k", wk)
    load_w("v", wv)
    msk_sb = singles.tile([128, 896], BF16)
    nc.sync.dma_start(out=msk_sb, in_=msk)
    wo_sb = singles.tile([128, D], BF16)
    nc.sync.dma_start(out=wo_sb, in_=wo)
    for g in range(2, NSTRIP):
        load_xt(g)

    qt_sb = singles.tile([128, NT], BF16)
    kt_sb = singles.tile([128, NT], BF16)
    v_sb = singles.tile([128, B * HLOC * KT_TILES, HD + 1], BF16)
    nc.vector.memset(v_sb[:, :, HD:HD + 1], 1.0)

    avf = {}  # strip -> assembled [128, 512] bf16 avT tile (both heads)

    def proj_units(g):
        gs = g * QSTRIP
        st = {}
        units = []

        def qk_mm(name, lo, hi, first, last, dst):
            def u():
                if first:
                    st[name] = psM.tile([128, QSTRIP], F32, tag="mm", name=f"ps_{name}")
                ps = st[name]
                for k in range(lo, hi):
                    nc.tensor.matmul(
                        ps, lhsT=w_sb[name][:, k, :],
                        rhs=xt_sb[:, k, gs:gs + QSTRIP],
                        start=(k == 0), stop=(k == 7))
                if last:
                    nc.vector.tensor_copy(dst[:, gs:gs + QSTRIP], ps)
            return u

        units.append(qk_mm("q", 0, 4, True, False, qt_sb))
        units.append(qk_mm("q", 4, 8, False, True, qt_sb))
        units.append(qk_mm("k", 0, 4, True, False, kt_sb))
        units.append(qk_mm("k", 4, 8, False, True, kt_sb))

        def v_mm(tt):
            b, j = divmod(g, 4)

            def u():
                if tt == 0:
                    st["v"] = psM.tile([128, QSTRIP], F32, tag="mm", name="ps_v")
                ps = st["v"]
                for k in range(8):
                    nc.tensor.matmul(
                        ps[:, tt * 128:(tt + 1) * 128],
                        lhsT=xt_sb[:, k, gs + tt * 128:gs + (tt + 1) * 128],
                        rhs=w_sb["v"][:, k, :],
                        start=(k == 0), stop=(k == 7))
                # both heads' v slices in one strided copy; idx h-stride = 16
                idx = b * HLOC * KT_TILES + 4 * j + tt
                nc.vector.tensor_copy(
                    v_sb[:, idx:idx + KT_TILES + 1:KT_TILES, 0:HD],
                    v_ps_view(ps, tt))
            return u

        def v_ps_view(ps, tt):
            return ps[:, tt * 128:(tt + 1) * 128].rearrange(
                "p (h d) -> p h d", h=2)

        for tt in range(4):
            units.append(v_mm(tt))
        return units

    def attn_units(g):
        b, j = divmod(g, 4)
        units = []
        st = {}

        def mk_pair(h, p):
            def u():
                if p == 0:
                    if h == 0:
                        avf[g] = sbp.tile([128, QSTRIP], BF16, tag="avf", name="avf")
                    st["av"] = psV.tile([HD + 1, QSTRIP], F32, tag="av", name="av_ps")
                av_ps = st["av"]
                hp = h * HD
                ntl = 4 * (j + 1)
                ts = (2 * p, 2 * p + 1)
                q0s = [max(0, 128 * (t - 4 * j)) for t in ts]
                sc_ps = psS.tile([128, 2, QSTRIP], F32, tag="sc", name="sc_ps")
                for i, t in enumerate(ts):
                    nc.tensor.matmul(
                        sc_ps[:, i, q0s[i]:],
                        lhsT=kt_sb[hp:hp + HD,
                                   b * S + t * 128: b * S + (t + 1) * 128],
                        rhs=qt_sb[hp:hp + HD,
                                  b * S + j * QSTRIP + q0s[i]:
                                  b * S + (j + 1) * QSTRIP],
                        start=True, stop=True)
                pexp = expp.tile([128, 2, QSTRIP], BF16, tag="pexp", name="pexp")
                if q0s[0] == q0s[1]:
                    # one wide exp over both kv tiles
                    nc.scalar.activation(
                        pexp[:, :, q0s[0]:], sc_ps[:, :, q0s[0]:],
                        mybir.ActivationFunctionType.Exp, scale=0.125)
                else:  # diagonal pair: exact valid ranges per tile
                    for i in range(2):
                        nc.scalar.activation(
                            pexp[:, i, q0s[i]:], sc_ps[:, i, q0s[i]:],
                            mybir.ActivationFunctionType.Exp, scale=0.125)
                for i, t in enumerate(ts):
                    r = t - 4 * j
                    if r >= 0:  # triangular mask on the diagonal 128-block
                        nc.vector.tensor_mul(
                            pexp[:, i, q0s[i]:q0s[i] + 128],
                            pexp[:, i, q0s[i]:q0s[i] + 128],
                            msk_sb[:, 384:512])
                    idx = (b * HLOC + h) * KT_TILES + t
                    nc.tensor.matmul(
                        av_ps[:, q0s[i]:], lhsT=v_sb[:, idx, :],
                        rhs=pexp[:, i, q0s[i]:],
                        start=(t == 0), stop=(t == ntl - 1))
            return u

        def mk_norm(h):
            def u():
                av_ps = st["av"]
                s_sb = sbp.tile([HD + 1, QSTRIP], F32, tag="s", name="s_sb")
                nc.vector.tensor_copy(s_sb, av_ps)
                s_dr = drp.tile([1, QSTRIP], F32, tag="sdr")
                nc.sync.dma_start(out=s_dr, in_=s_sb[HD:HD + 1, :])
                rb = sbp.tile([HD, QSTRIP], F32, tag="rb")
                nc.sync.dma_start(
                    out=rb, in_=s_dr[0, :].partition_broadcast(HD))
                nc.vector.reciprocal_approx_fast(rb, rb)
                avh = sbp.tile([HD, QSTRIP], BF16, tag="avh")
                nc.vector.tensor_mul(avh, s_sb[0:HD, :], rb)
                nc.sync.dma_start(out=avf[g][h * HD:(h + 1) * HD, :], in_=avh)
            return u

        for h in range(HLOC):
            for p in range(2 * (j + 1)):
                units.append(mk_pair(h, p))
            units.append(mk_norm(h))
        return units

    def out_units(g):
        gs = g * QSTRIP
        units = []

        def mk(tt):
            def u():
                ob = outp.tile([128, D], F32, tag="ob")
                for n in range(2):
                    op_ps = psM.tile([128, 512], F32, tag="mm", name="op_ps")
                    nc.tensor.matmul(
                        op_ps, lhsT=avf[g][:, tt * 128:(tt + 1) * 128],
                        rhs=wo_sb[:, n * 512:(n + 1) * 512],
                        start=True, stop=True)
                    nc.vector.tensor_copy(ob[:, n * 512:(n + 1) * 512], op_ps)
                nc.sync.dma_start(
                    out=out[gs + tt * 128: gs + (tt + 1) * 128, :], in_=ob)
            return u
        for tt in range(4):
            units.append(mk(tt))
        return units

    order = [0, 1, 2, 3, 7, 6, 5, 4]
    proj_fill = {0: [1], 1: [2], 2: [3, 4], 3: [5, 6, 7],
                 7: [], 6: [], 5: [], 4: []}
    for u in proj_units(0):
        u()
    prev = None
    for g in order:
        fill = []
        for pg in proj_fill[g]:
            fill += proj_units(pg)
        if prev is not None:
            fill += out_units(prev)
        _interleave(attn_units(g), fill)
        prev = g
    for u in out_units(order[-1]):
        u()


_CACHED_NC = None


def build_module():
    global _CACHED_NC
    if _CACHED_NC is None:
        nc = bacc.Bacc("TRN2", debug=False)
        with tile.TileContext(nc) as tc:
            with ExitStack() as ctx:
                _build_kernel(ctx, tc)
        nc.compile()
        _CACHED_NC = nc
    return _CACHED_NC


def make_in_maps(x, Wq, Wk, Wv, Wo):
    x = np.asarray(x, np.float32)
    xT = np.ascontiguousarray(x.reshape(NT, D).T).astype(NPBF16)
    # sliding causal mask: keep (c - 384) >= i
    i = np.arange(128)[:, None]
    c = np.arange(896)[None, :]
    msk = ((c - 384) >= i).astype(NPBF16)
    in_maps = []
    for core in range(NCORES):
        cs = slice(core * CW, (core + 1) * CW)
        in_maps.append({
            "xt": xT,
            "wq": np.asarray(Wq, np.float32)[:, cs].astype(NPBF16),
            "wk": np.asarray(Wk, np.float32)[:, cs].astype(NPBF16),
            "wv": np.asarray(Wv, np.float32)[:, cs].astype(NPBF16),
            "wo": np.ascontiguousarray(np.asarray(Wo, np.float32)[cs, :]).astype(NPBF16),
            "msk": msk,
        })
    return in_maps


def kernel(x, Wq, bq, Wk, bk, Wv, bv, Wo, bo):
    for b_ in (bq, bk, bv, bo):
        assert np.count_nonzero(np.asarray(b_)) == 0, "nonzero biases unsupported"
    nc = build_module()
    in_maps = make_in_maps(x, Wq, Wk, Wv, Wo)
    res = run_bass_kernel_spmd(nc, in_maps, core_ids=list(range(NCORES)))
    partials = [res.results[c]["out"] for c in range(NCORES)]
    total = np.sum(np.stack(partials, 0), axis=0, dtype=np.float32)
    return total.reshape(B, S, D)
